# revision 2
# baseline (speedup 1.0000x reference)
"""nn_MAB kernel: 8-core data-parallel Bass/Tile implementation for TRN2.

The axon host<->device link (~40MB/s) dominates wall time, so inputs are
quantized to int8 on host (per-tensor scale) and the output returns as bf16.
Weights are host-folded (G-form not needed: a/b rank-16 transforms keep the
contraction at 640; 1/(inter*T) folded into Wa, gamma into Wd, biases are
structurally zero) and embedded in the NEFF as inline consts.

Device dataflow per core (32 batches, 8 groups of 4):
  k8 -> dequant bf16 -> PE-transpose to t-major d-chunks XD (p = dt*64+c).
  Per gcn, per subset: a/b transforms (full-width), M^T = b^T a per group
  (block-diag, off-diag garbage masked), free-dim softmax (no max-sub),
  +PA^T, PE-transpose -> S block-diag lhsT.  Then per group: U = Wd'
  transform, z = S^T U with beta seeded into PSUM via a K=1 ones matmul,
  y = relu(z + x) with x re-dequantized per group.
  Attention: c-major PE-transposes of Kf/Q, per-head scores (contraction
  pieces across 128-chunks), masked softmax, attn^T via PE, O = Q + attn@Vf.
"""
import hashlib
import numpy as np
import ml_dtypes

import concourse.bass as bass
from concourse import bacc, mybir, bass_utils, masks
from concourse.tile import TileContext

BF = mybir.dt.bfloat16
F32 = mybir.dt.float32
I8 = mybir.dt.int8
EXP = mybir.ActivationFunctionType.Exp
RELU = mybir.ActivationFunctionType.Relu
COPY = mybir.ActivationFunctionType.Copy

NCORES = 8
BC = 32
NG = 8
GB = 4
D, C, T, INTER = 2560, 64, 40, 16
VK, VO, NH, DS = 25, 10, 8, 320


def _pieces(lo, hi, step=128):
    out = []
    while lo < hi:
        ci, off = lo // step, lo % step
        ln = min(step - off, hi - lo)
        out.append((ci, off, ln))
        lo += ln
    return out


def _build(consts, num_devices):
    nc = bacc.Bacc("TRN2", target_bir_lowering=False, debug=False,
                   enable_asserts=False, num_devices=num_devices)
    k8_d = nc.dram_tensor("k8", (BC * VK, D), I8, kind="ExternalInput").ap()
    q8_d = nc.dram_tensor("q8", (BC * VO, D), I8, kind="ExternalInput").ap()
    out_d = nc.dram_tensor("outp", (BC * VO, D), BF, kind="ExternalOutput").ap()
    cap = {k: nc.inline_tensor(v, name="c_" + k).ap() for k, v in consts.items()}

    with TileContext(nc) as tc:
        with tc.tile_pool(name="persist", bufs=1) as persist, \
             tc.tile_pool(name="work", bufs=2) as work, \
             tc.tile_pool(name="ps", bufs=3, space="PSUM") as psp, \
             tc.tile_pool(name="psz", bufs=4, space="PSUM") as psz:

            ident = persist.tile([128, 128], BF, tag="ident", name="ident")
            masks.make_identity(nc, ident[:])

            def load_const(name, shape, dtype=BF):
                t = persist.tile(list(shape), dtype, tag="c_" + name)
                nc.sync.dma_start(t[:], cap[name])
                return t

            skc = load_const("sk", (128, 1), F32)
            sqc = load_const("sq", (128, 1), F32)
            ones_t = persist.tile([1, 128], BF, tag="ones", name="ones")
            nc.vector.memset(ones_t[:], 1.0)

            W = {}
            for g in ("k", "v", "o"):
                for i in range(3):
                    W[f"wab_{g}{i}"] = load_const(f"wab_{g}{i}", (128, 64))
                W[f"wd_{g}"] = load_const(f"wd_{g}", (128, 384))
                gv = VK if g != "o" else VO
                W[f"pam_{g}"] = load_const(f"pam_{g}", (GB * gv, 3, GB * gv))
                W[f"mask_{g}"] = load_const(f"mask_{g}", (GB * gv, GB * gv))
                W[f"beta_{g}"] = load_const(f"beta_{g}", (1, D))
            mask_att = load_const("mask_att", (GB * VO, GB * VK))

            # repacked 3-col layout for 40-row (VO) natural tensors
            def nsl(tile, g, R):
                return tile[64 * (g % 2):64 * (g % 2) + R, g // 2]

            On = persist.tile([104, 4, C, T], BF, tag="On", name="On")

            def dequant_k_group(pool, g, tag):
                st = work.tile([GB * VK, D], I8, tag="stg_k", name="stg_k")
                nc.sync.dma_start(st[:], k8_d[g * GB * VK:(g + 1) * GB * VK, :])
                xr = pool.tile([GB * VK, D], BF, tag=tag, name="kres")
                nc.vector.tensor_scalar_mul(xr[:], st[:], skc[:GB * VK])
                return xr

            def run_gcn(gname, XDl, Vn, res_ap, sink):
                """one unit_gcn.  XDl: 20 t-major chunks (128, BC*Vn).
                res_ap(g) -> (R, D) residual AP.  sink(g, n, yt) stores output."""
                R = GB * Vn
                W3 = BC * Vn
                with tc.tile_pool(name="gcn_" + gname, bufs=1) as gp, \
                     tc.tile_pool(name="gw_" + gname, bufs=1) as gw:
                    Sb = [[gp.tile([R, R], BF, tag=f"sb{i}_{g}", name=f"sb{i}_{g}") for g in range(NG)]
                          for i in range(3)]
                    nh2 = 2 if Vn == VK else 1
                    NW = W3 // nh2
                    for i in range(3):
                      for h2 in range(nh2):
                        Ad = [gw.tile([128, NW], BF, tag=f"ad{q}", name=f"ad{q}") for q in range(5)]
                        Bd = [gw.tile([128, NW], BF, tag=f"bd{q}", name=f"bd{q}") for q in range(5)]
                        wab = W[f"wab_{gname}{i}"]
                        for k in range(20):
                            pab = psp.tile([64, NW], F32, tag="ps", name="ps")
                            nc.tensor.matmul(pab[:], wab[:],
                                             XDl[k][:, h2 * NW:(h2 + 1) * NW],
                                             start=True, stop=True)
                            q, r0 = k // 4, 32 * (k % 4)
                            nc.vector.tensor_copy(Ad[q][r0:r0 + 32, :], pab[0:32, :])
                            nc.vector.tensor_copy(Bd[q][r0:r0 + 32, :], pab[32:64, :])
                        for g in range(h2 * NG // nh2, (h2 + 1) * NG // nh2):
                            gsl = slice(g * R - h2 * NW, (g + 1) * R - h2 * NW)
                            mt = psz.tile([R, R], F32, tag="psz", name="psz")
                            for q in range(5):
                                nc.tensor.matmul(mt[:], Bd[q][:, gsl], Ad[q][:, gsl],
                                                 start=(q == 0), stop=(q == 4))
                            E = work.tile([R, GB, Vn], BF, tag="et", name="et")
                            nc.scalar.activation(E[:].rearrange("p a b -> p (a b)"),
                                                 mt[:], EXP)
                            red = work.tile([R, GB], F32, tag="red", name="red")
                            nc.vector.tensor_reduce(red[:], E[:], mybir.AxisListType.X,
                                                    mybir.AluOpType.add)
                            rec = work.tile([R, GB], F32, tag="rec", name="rec")
                            nc.vector.reciprocal(rec[:], red[:])
                            ST = work.tile([R, R], BF, tag="st", name="st")
                            for j in range(GB):
                                nc.vector.tensor_scalar_mul(
                                    ST[:, j * Vn:(j + 1) * Vn], E[:, j], rec[:, j:j + 1])
                            nc.vector.tensor_mul(ST[:], ST[:], W[f"mask_{gname}"][:])
                            nc.vector.tensor_add(ST[:], ST[:], W[f"pam_{gname}"][:, i])
                            pst = psp.tile([R, R], BF, tag="ps", name="ps")
                            nc.tensor.transpose(pst[:], ST[:], ident[:R, :R])
                            nc.scalar.activation(Sb[i][g][:], pst[:], COPY)
                    for g in range(NG):
                        gs = slice(g * R, (g + 1) * R)
                        Ug = [gw.tile([R, C, T], BF, tag=f"ug{i}", name=f"ug{i}") for i in range(3)]
                        for k in range(20):
                            pu = psp.tile([R, 384], F32, tag="ps", name="ps")
                            nc.tensor.matmul(pu[:], XDl[k][:, gs], W[f"wd_{gname}"][:],
                                             start=True, stop=True)
                            for i in range(3):
                                dst = Ug[i][:, :, 2 * k:2 * k + 2].transpose([0, 2, 1])
                                src = pu[:, i * 128:i * 128 + 128] \
                                    .rearrange("p (a b) -> p a b", a=2)
                                nc.vector.tensor_copy(dst, src)
                        xres = res_ap(g)
                        for n in range(5):
                            ns = slice(n * 512, (n + 1) * 512)
                            pz = psz.tile([R, 512], F32, tag="psz", name="psz")
                            nc.tensor.matmul(pz[:], ones_t[:, :R],
                                             W[f"beta_{gname}"][:, ns],
                                             start=True, stop=False)
                            for i in range(3):
                                nc.tensor.matmul(
                                    pz[:], Sb[i][g][:],
                                    Ug[i][:].rearrange("p c t -> p (c t)")[:, ns],
                                    start=False, stop=(i == 2))
                            yt = work.tile([R, 512], BF, tag="yt", name="yt")
                            nc.vector.tensor_add(yt[:], pz[:], xres[:, ns])
                            sink(g, n, yt)

            with tc.tile_pool(name="kv", bufs=1) as kvp:
                Vf = kvp.tile([GB * VK, NG, C, T], BF, tag="Vf", name="Vf")
                KfdC = [kvp.tile([128, BC * VK], BF, tag=f"kfd{k}", name=f"kfd{k}")
                        for k in range(20)]

                with tc.tile_pool(name="xd", bufs=1) as xdp:
                    XD = [xdp.tile([128, BC * VK], BF, tag=f"xd{k}", name=f"xd{k}")
                          for k in range(20)]
                    for g in range(NG):
                        xr = dequant_k_group(work, g, "kres")
                        xv = xr[:].rearrange("p (c t) -> p c t", c=C)
                        for k in range(20):
                            pt = psp.tile([128, GB * VK], BF, tag="ps", name="ps")
                            for dt in range(2):
                                nc.tensor.transpose(pt[64 * dt:64 * dt + 64, :],
                                                    xv[:, :, 2 * k + dt],
                                                    ident[:GB * VK, :GB * VK])
                            nc.vector.tensor_copy(
                                XD[k][:, g * GB * VK:(g + 1) * GB * VK], pt[:])

                    def kres(g):
                        st = work.tile([GB * VK, D], I8, tag="stg_k", name="stg_k")
                        nc.sync.dma_start(st[:], k8_d[g * GB * VK:(g + 1) * GB * VK, :])
                        xr = work.tile([GB * VK, D], BF, tag="kres", name="kres")
                        nc.vector.tensor_scalar_mul(xr[:], st[:], skc[:GB * VK])
                        return xr[:]

                    def k_sink(g, n, yt):
                        gs = slice(g * GB * VK, (g + 1) * GB * VK)
                        tk = work.tile([GB * VK, 512], BF, tag="tk", name="tk")
                        nc.scalar.activation(tk[:], yt[:], RELU)
                        for j in range(4):
                            ptk = psp.tile([128, GB * VK], BF, tag="ps", name="ps")
                            nc.tensor.transpose(ptk[:], tk[:, 128 * j:128 * (j + 1)],
                                                ident[:GB * VK, :GB * VK])
                            nc.vector.tensor_copy(KfdC[4 * n + j][:, gs], ptk[:])

                    def v_sink(g, n, yt):
                        ns = slice(n * 512, (n + 1) * 512)
                        nc.scalar.activation(
                            Vf[:, g].rearrange("p c t -> p (c t)")[:, ns], yt[:], RELU)

                    run_gcn("k", XD, VK, kres, k_sink)
                    run_gcn("v", XD, VK, kres, v_sink)

                # ---- attention ----
                with tc.tile_pool(name="attn", bufs=1) as ap_:
                    Qn = ap_.tile([104, 4, C, T], BF, tag="Qn", name="Qn")
                    for g in range(NG):
                        stq = work.tile([GB * VO, D], I8, tag="stg_k", name="stg_q")
                        nc.sync.dma_start(stq[:], q8_d[g * GB * VO:(g + 1) * GB * VO, :])
                        nc.vector.tensor_scalar_mul(
                            nsl(Qn, g, GB * VO).rearrange("p c t -> p (c t)"),
                            stq[:], sqc[:GB * VO])
                    QdC = [ap_.tile([128, BC * VO], BF, tag=f"qd{k}", name=f"qd{k}")
                           for k in range(20)]
                    for k in range(20):
                        for g in range(NG):
                            pt2 = psp.tile([128, GB * VO], BF, tag="ps", name="ps")
                            b0 = 64 * (g % 2)
                            nc.tensor.transpose(
                                pt2[:], nsl(Qn, g, GB * VO)
                                .rearrange("p c t -> p (c t)")[:, 128 * k:128 * (k + 1)],
                                ident[b0:b0 + GB * VO, b0:b0 + GB * VO])
                            nc.vector.tensor_copy(
                                QdC[k][:, g * GB * VO:(g + 1) * GB * VO], pt2[:])

                    sc_scale = float(1.0 / np.sqrt(np.float32(D)))
                    for g in range(NG):
                        qs = slice(g * GB * VO, (g + 1) * GB * VO)
                        ks = slice(g * GB * VK, (g + 1) * GB * VK)
                        qn_g = nsl(Qn, g, GB * VO).rearrange("p c t -> p (c t)")
                        on_g = nsl(On, g, GB * VO).rearrange("p c t -> p (c t)")
                        vf_g = Vf[:, g].rearrange("p c t -> p (c t)")
                        for h in range(NH):
                            pcs = _pieces(h * DS, (h + 1) * DS)
                            psc = psz.tile([GB * VO, GB * VK], F32, tag="psz", name="psz")
                            for pi, (ci, off, ln) in enumerate(pcs):
                                nc.tensor.matmul(psc[:], QdC[ci][off:off + ln, qs],
                                                 KfdC[ci][off:off + ln, ks],
                                                 start=(pi == 0),
                                                 stop=(pi == len(pcs) - 1))
                            E = work.tile([GB * VO, GB, VK], BF, tag="eat", name="eat")
                            nc.scalar.activation(E[:].rearrange("p a b -> p (a b)"),
                                                 psc[:], EXP, scale=sc_scale)
                            red = work.tile([GB * VO, GB], F32, tag="reda", name="reda")
                            nc.vector.tensor_reduce(red[:], E[:], mybir.AxisListType.X,
                                                    mybir.AluOpType.add)
                            rec = work.tile([GB * VO, GB], F32, tag="reca", name="reca")
                            nc.vector.reciprocal(rec[:], red[:])
                            AT = work.tile([GB * VO, GB * VK], BF, tag="at", name="at")
                            for j in range(GB):
                                nc.vector.tensor_scalar_mul(
                                    AT[:, j * VK:(j + 1) * VK], E[:, j], rec[:, j:j + 1])
                            nc.vector.tensor_mul(AT[:], AT[:], mask_att[:])
                            pat = psp.tile([GB * VK, GB * VO], BF, tag="ps", name="ps")
                            nc.tensor.transpose(pat[:], AT[:], ident[:GB * VO, :GB * VO])
                            ATT = work.tile([GB * VK, GB * VO], BF, tag="att", name="att")
                            nc.scalar.activation(ATT[:], pat[:], COPY)
                            pov = psz.tile([GB * VO, DS], F32, tag="psz", name="psz")
                            nc.tensor.matmul(pov[:], ATT[:],
                                             vf_g[:, h * DS:(h + 1) * DS],
                                             start=True, stop=True)
                            nc.vector.tensor_add(on_g[:, h * DS:(h + 1) * DS],
                                                 pov[:], qn_g[:, h * DS:(h + 1) * DS])

            # ---- fco on On ----
            with tc.tile_pool(name="fco", bufs=1) as fp:
                OnD = [fp.tile([128, BC * VO], BF, tag=f"ond{k}", name=f"ond{k}") for k in range(20)]
                for k in range(20):
                    for g in range(NG):
                        pt = psp.tile([128, GB * VO], BF, tag="ps", name="ps")
                        b0 = 64 * (g % 2)
                        for dt in range(2):
                            nc.tensor.transpose(pt[64 * dt:64 * dt + 64, :],
                                                nsl(On, g, GB * VO)[:, :, 2 * k + dt],
                                                ident[b0:b0 + GB * VO, b0:b0 + GB * VO])
                        nc.vector.tensor_copy(
                            OnD[k][:, g * GB * VO:(g + 1) * GB * VO], pt[:])

                def ores(g):
                    return nsl(On, g, GB * VO).rearrange("p c t -> p (c t)")

                Og = fp.tile([104, 4, C, T], BF, tag="Og", name="Og")

                def o_sink(g, n, yt):
                    ns = slice(n * 512, (n + 1) * 512)
                    nc.scalar.activation(
                        nsl(Og, g, GB * VO).rearrange("p c t -> p (c t)")[:, ns],
                        yt[:], RELU)

                run_gcn("o", OnD, VO, ores, o_sink)

                Fo = fp.tile([104, 4, C, T], BF, tag="Fo", name="Fo")
                for g in range(NG):
                    fo_g = nsl(Fo, g, GB * VO).rearrange("p c t -> p (c t)")
                    nc.vector.tensor_add(fo_g, ores(g),
                                         nsl(Og, g, GB * VO)
                                         .rearrange("p c t -> p (c t)"))
                    nc.sync.dma_start(out_d[g * GB * VO:(g + 1) * GB * VO, :], fo_g)
    nc.finalize()
    return nc


def _prep_consts(inp, sK, sQ):
    bf = ml_dtypes.bfloat16
    consts = {
        "sk": np.full((128, 1), sK, np.float32),
        "sq": np.full((128, 1), sQ, np.float32),
        "mask_att": np.kron(np.eye(GB, dtype=np.float32),
                            np.ones((VO, VK), np.float32)).astype(bf),
    }
    for gname, pref, Vn in (("k", "fck", VK), ("v", "fcv", VK), ("o", "fco", VO)):
        Wa = np.asarray(inp[pref + "_Wa"], np.float32) / (INTER * T)
        Wb = np.asarray(inp[pref + "_Wb"], np.float32)
        Wd = np.asarray(inp[pref + "_Wd"], np.float32)
        PA = np.asarray(inp[pref + "_PA"], np.float32)
        gam = np.asarray(inp[pref + "_gamma"], np.float32)
        bet = np.asarray(inp[pref + "_beta"], np.float32)
        gsc = gam / np.sqrt(np.float32(1.0 + 1e-5))
        Wdf = Wd * gsc[None, :, None]
        for i in range(3):
            wab = np.zeros((128, 64), np.float32)
            for dt in range(2):
                wab[dt * 64:dt * 64 + 64, dt * 16:dt * 16 + 16] = Wa[i].T
                wab[dt * 64:dt * 64 + 64, 32 + dt * 16:32 + dt * 16 + 16] = Wb[i].T
            consts[f"wab_{gname}{i}"] = wab.astype(bf)
        wd3 = np.zeros((128, 384), np.float32)
        for i in range(3):
            for dt in range(2):
                wd3[dt * 64:dt * 64 + 64,
                    i * 128 + dt * 64:i * 128 + dt * 64 + 64] = Wdf[i].T
        consts[f"wd_{gname}"] = wd3.astype(bf)
        mask = np.kron(np.eye(GB, dtype=np.float32), np.ones((Vn, Vn), np.float32))
        consts[f"mask_{gname}"] = mask.astype(bf)
        pam = np.stack([np.tile(PA[i].T, (GB, GB)) * mask for i in range(3)], axis=1)
        consts[f"pam_{gname}"] = pam.astype(bf)
        consts[f"beta_{gname}"] = np.repeat(bet, T).reshape(1, D).astype(bf)
    return consts


_cache = {}


def kernel(**inputs):
    Q = np.asarray(inputs["Q"], np.float32)
    K = np.asarray(inputs["K"], np.float32)
    B = Q.shape[0]
    sQ = float(np.abs(Q).max()) / 127.0
    sK = float(np.abs(K).max()) / 127.0
    Q8 = np.clip(np.rint(Q * (1.0 / sQ)), -127, 127).astype(np.int8)
    K8 = np.clip(np.rint(K * (1.0 / sK)), -127, 127).astype(np.int8)

    wkeys = sorted(k for k in inputs if k.startswith(("fck_", "fcv_", "fco_")))
    dig = hashlib.sha256()
    for k in wkeys:
        dig.update(np.ascontiguousarray(np.asarray(inputs[k])).tobytes())
    dig.update(np.float32(sQ).tobytes())
    dig.update(np.float32(sK).tobytes())
    key = dig.hexdigest()
    if key not in _cache:
        consts = _prep_consts(inputs, sK, sQ)
        _cache.clear()
        _cache[key] = _build(consts, NCORES)
    nc = _cache[key]

    K8s = K8.reshape(NCORES, BC * VK, D)
    Q8s = Q8.reshape(NCORES, BC * VO, D)
    in_maps = [{"k8": K8s[c], "q8": Q8s[c]} for c in range(NCORES)]
    res = bass_utils.run_bass_kernel_spmd(nc, in_maps, core_ids=list(range(NCORES)))
    out = np.stack([np.asarray(res.results[c]["outp"]) for c in range(NCORES)])
    return out.reshape(B, VO, D).astype(np.float32)


# revision 4
# speedup vs baseline: 2.4530x; 2.4530x over previous
"""nn_MAB kernel: 8-core data-parallel Bass/Tile implementation for TRN2.

The axon host<->device link (~40MB/s) dominates wall time, so inputs are
quantized to int8 on host (per-tensor scale) and the output returns as bf16.
Weights are host-folded (G-form not needed: a/b rank-16 transforms keep the
contraction at 640; 1/(inter*T) folded into Wa, gamma into Wd, biases are
structurally zero) and embedded in the NEFF as inline consts.

Device dataflow per core (32 batches, 8 groups of 4):
  k8 -> dequant bf16 -> PE-transpose to t-major d-chunks XD (p = dt*64+c).
  Per gcn, per subset: a/b transforms (full-width), M^T = b^T a per group
  (block-diag, off-diag garbage masked), free-dim softmax (no max-sub),
  +PA^T, PE-transpose -> S block-diag lhsT.  Then per group: U = Wd'
  transform, z = S^T U with beta seeded into PSUM via a K=1 ones matmul,
  y = relu(z + x) with x re-dequantized per group.
  Attention: c-major PE-transposes of Kf/Q, per-head scores (contraction
  pieces across 128-chunks), masked softmax, attn^T via PE, O = Q + attn@Vf.
"""
import hashlib
import numpy as np
import ml_dtypes

import concourse.bass as bass
from concourse import bacc, mybir, bass_utils, masks
from concourse.tile import TileContext

BF = mybir.dt.bfloat16
F32 = mybir.dt.float32
I8 = mybir.dt.int8
EXP = mybir.ActivationFunctionType.Exp
RELU = mybir.ActivationFunctionType.Relu
COPY = mybir.ActivationFunctionType.Copy

NCORES = 8
BC = 32
NG = 8
GB = 4
D, C, T, INTER = 2560, 64, 40, 16
VK, VO, NH, DS = 25, 10, 8, 320


def _pieces(lo, hi, step=128):
    out = []
    while lo < hi:
        ci, off = lo // step, lo % step
        ln = min(step - off, hi - lo)
        out.append((ci, off, ln))
        lo += ln
    return out


def _build(consts, num_devices):
    nc = bacc.Bacc("TRN2", target_bir_lowering=False, debug=False,
                   enable_asserts=False, num_devices=num_devices)
    k8_d = nc.dram_tensor("k8", (BC * VK, D), I8, kind="ExternalInput").ap()
    q8_d = nc.dram_tensor("q8", (BC * VO, D), I8, kind="ExternalInput").ap()
    out_d = nc.dram_tensor("outp", (BC * VO, D), BF, kind="ExternalOutput").ap()
    cap = {k: nc.inline_tensor(v, name="c_" + k).ap() for k, v in consts.items()}

    with TileContext(nc) as tc:
        with tc.tile_pool(name="persist", bufs=1) as persist, \
             tc.tile_pool(name="work", bufs=2) as work, \
             tc.tile_pool(name="ps", bufs=3, space="PSUM") as psp, \
             tc.tile_pool(name="psz", bufs=4, space="PSUM") as psz:

            ident = persist.tile([128, 128], BF, tag="ident", name="ident")
            masks.make_identity(nc, ident[:])

            def load_const(name, shape, dtype=BF):
                t = persist.tile(list(shape), dtype, tag="c_" + name)
                nc.sync.dma_start(t[:], cap[name])
                return t

            skc = load_const("sk", (128, 1), F32)
            sqc = load_const("sq", (128, 1), F32)
            ones_t = persist.tile([1, 128], BF, tag="ones", name="ones")
            nc.vector.memset(ones_t[:], 1.0)

            W = {}
            for g in ("k", "v", "o"):
                for i in range(3):
                    W[f"wab_{g}{i}"] = load_const(f"wab_{g}{i}", (128, 64))
                W[f"wd_{g}"] = load_const(f"wd_{g}", (128, 384))
                gv = VK if g != "o" else VO
                W[f"pam_{g}"] = load_const(f"pam_{g}", (GB * gv, 3, GB * gv))
                W[f"mask_{g}"] = load_const(f"mask_{g}", (GB * gv, GB * gv))
                W[f"beta_{g}"] = load_const(f"beta_{g}", (1, D))
            mask_att = load_const("mask_att", (GB * VO, GB * VK))

            # repacked 3-col layout for 40-row (VO) natural tensors
            def nsl(tile, g, R):
                return tile[64 * (g % 2):64 * (g % 2) + R, g // 2]

            On = persist.tile([104, 4, C, T], BF, tag="On", name="On")

            def dequant_k_group(pool, g, tag):
                st = work.tile([GB * VK, D], I8, tag="stg_k", name="stg_k")
                nc.sync.dma_start(st[:], k8_d[g * GB * VK:(g + 1) * GB * VK, :])
                xr = pool.tile([GB * VK, D], BF, tag=tag, name="kres")
                nc.vector.tensor_scalar_mul(xr[:], st[:], skc[:GB * VK])
                return xr

            def run_gcn(gname, XDl, Vn, res_ap, sink):
                """one unit_gcn.  XDl: 20 t-major chunks (128, BC*Vn).
                res_ap(g) -> (R, D) residual AP.  sink(g, n, yt) stores output."""
                R = GB * Vn
                W3 = BC * Vn
                with tc.tile_pool(name="gcn_" + gname, bufs=1) as gp, \
                     tc.tile_pool(name="gw_" + gname, bufs=1) as gw:
                    Sb = [[gp.tile([R, R], BF, tag=f"sb{i}_{g}", name=f"sb{i}_{g}") for g in range(NG)]
                          for i in range(3)]
                    nh2 = 2 if Vn == VK else 1
                    NW = W3 // nh2
                    for i in range(3):
                      for h2 in range(nh2):
                        Ad = [gw.tile([128, NW], BF, tag=f"ad{q}", name=f"ad{q}") for q in range(5)]
                        Bd = [gw.tile([128, NW], BF, tag=f"bd{q}", name=f"bd{q}") for q in range(5)]
                        wab = W[f"wab_{gname}{i}"]
                        for k in range(20):
                            pab = psp.tile([64, NW], F32, tag="ps", name="ps")
                            nc.tensor.matmul(pab[:], wab[:],
                                             XDl[k][:, h2 * NW:(h2 + 1) * NW],
                                             start=True, stop=True)
                            q, r0 = k // 4, 32 * (k % 4)
                            nc.vector.tensor_copy(Ad[q][r0:r0 + 32, :], pab[0:32, :])
                            nc.vector.tensor_copy(Bd[q][r0:r0 + 32, :], pab[32:64, :])
                        for g in range(h2 * NG // nh2, (h2 + 1) * NG // nh2):
                            gsl = slice(g * R - h2 * NW, (g + 1) * R - h2 * NW)
                            mt = psz.tile([R, R], F32, tag="psz", name="psz")
                            for q in range(5):
                                nc.tensor.matmul(mt[:], Bd[q][:, gsl], Ad[q][:, gsl],
                                                 start=(q == 0), stop=(q == 4))
                            E = work.tile([R, GB, Vn], BF, tag="et", name="et")
                            nc.scalar.activation(E[:].rearrange("p a b -> p (a b)"),
                                                 mt[:], EXP)
                            red = work.tile([R, GB], F32, tag="red", name="red")
                            nc.vector.tensor_reduce(red[:], E[:], mybir.AxisListType.X,
                                                    mybir.AluOpType.add)
                            rec = work.tile([R, GB], F32, tag="rec", name="rec")
                            nc.vector.reciprocal(rec[:], red[:])
                            ST = work.tile([R, R], BF, tag="st", name="st")
                            for j in range(GB):
                                nc.vector.tensor_scalar_mul(
                                    ST[:, j * Vn:(j + 1) * Vn], E[:, j], rec[:, j:j + 1])
                            nc.vector.tensor_mul(ST[:], ST[:], W[f"mask_{gname}"][:])
                            nc.vector.tensor_add(ST[:], ST[:], W[f"pam_{gname}"][:, i])
                            pst = psp.tile([R, R], BF, tag="ps", name="ps")
                            nc.tensor.transpose(pst[:], ST[:], ident[:R, :R])
                            nc.scalar.activation(Sb[i][g][:], pst[:], COPY)
                    for g in range(NG):
                        gs = slice(g * R, (g + 1) * R)
                        Ug = [gw.tile([R, C, T], BF, tag=f"ug{i}", name=f"ug{i}") for i in range(3)]
                        for k in range(20):
                            pu = psp.tile([R, 384], F32, tag="ps", name="ps")
                            nc.tensor.matmul(pu[:], XDl[k][:, gs], W[f"wd_{gname}"][:],
                                             start=True, stop=True)
                            for i in range(3):
                                dst = Ug[i][:, :, 2 * k:2 * k + 2].transpose([0, 2, 1])
                                src = pu[:, i * 128:i * 128 + 128] \
                                    .rearrange("p (a b) -> p a b", a=2)
                                nc.vector.tensor_copy(dst, src)
                        xres = res_ap(g)
                        for n in range(5):
                            ns = slice(n * 512, (n + 1) * 512)
                            pz = psz.tile([R, 512], F32, tag="psz", name="psz")
                            nc.tensor.matmul(pz[:], ones_t[:, :R],
                                             W[f"beta_{gname}"][:, ns],
                                             start=True, stop=False)
                            for i in range(3):
                                nc.tensor.matmul(
                                    pz[:], Sb[i][g][:],
                                    Ug[i][:].rearrange("p c t -> p (c t)")[:, ns],
                                    start=False, stop=(i == 2))
                            yt = work.tile([R, 512], BF, tag="yt", name="yt")
                            nc.vector.tensor_add(yt[:], pz[:], xres[:, ns])
                            sink(g, n, yt)

            with tc.tile_pool(name="kv", bufs=1) as kvp:
                Vf = kvp.tile([GB * VK, NG, C, T], BF, tag="Vf", name="Vf")
                KfdC = [kvp.tile([128, BC * VK], BF, tag=f"kfd{k}", name=f"kfd{k}")
                        for k in range(20)]

                with tc.tile_pool(name="xd", bufs=1) as xdp:
                    XD = [xdp.tile([128, BC * VK], BF, tag=f"xd{k}", name=f"xd{k}")
                          for k in range(20)]
                    for g in range(NG):
                        xr = dequant_k_group(work, g, "kres")
                        xv = xr[:].rearrange("p (c t) -> p c t", c=C)
                        for k in range(20):
                            pt = psp.tile([128, GB * VK], BF, tag="ps", name="ps")
                            for dt in range(2):
                                nc.tensor.transpose(pt[64 * dt:64 * dt + 64, :],
                                                    xv[:, :, 2 * k + dt],
                                                    ident[:GB * VK, :GB * VK])
                            nc.vector.tensor_copy(
                                XD[k][:, g * GB * VK:(g + 1) * GB * VK], pt[:])

                    def kres(g):
                        st = work.tile([GB * VK, D], I8, tag="stg_k", name="stg_k")
                        nc.sync.dma_start(st[:], k8_d[g * GB * VK:(g + 1) * GB * VK, :])
                        xr = work.tile([GB * VK, D], BF, tag="kres", name="kres")
                        nc.vector.tensor_scalar_mul(xr[:], st[:], skc[:GB * VK])
                        return xr[:]

                    def k_sink(g, n, yt):
                        gs = slice(g * GB * VK, (g + 1) * GB * VK)
                        tk = work.tile([GB * VK, 512], BF, tag="tk", name="tk")
                        nc.scalar.activation(tk[:], yt[:], RELU)
                        for j in range(4):
                            ptk = psp.tile([128, GB * VK], BF, tag="ps", name="ps")
                            nc.tensor.transpose(ptk[:], tk[:, 128 * j:128 * (j + 1)],
                                                ident[:GB * VK, :GB * VK])
                            nc.vector.tensor_copy(KfdC[4 * n + j][:, gs], ptk[:])

                    def v_sink(g, n, yt):
                        ns = slice(n * 512, (n + 1) * 512)
                        nc.scalar.activation(
                            Vf[:, g].rearrange("p c t -> p (c t)")[:, ns], yt[:], RELU)

                    run_gcn("k", XD, VK, kres, k_sink)
                    run_gcn("v", XD, VK, kres, v_sink)

                # ---- attention ----
                with tc.tile_pool(name="attn", bufs=1) as ap_:
                    Qn = ap_.tile([104, 4, C, T], BF, tag="Qn", name="Qn")
                    for g in range(NG):
                        stq = work.tile([GB * VO, D], I8, tag="stg_k", name="stg_q")
                        nc.sync.dma_start(stq[:], q8_d[g * GB * VO:(g + 1) * GB * VO, :])
                        nc.vector.tensor_scalar_mul(
                            nsl(Qn, g, GB * VO).rearrange("p c t -> p (c t)"),
                            stq[:], sqc[:GB * VO])
                    QdC = [ap_.tile([128, BC * VO], BF, tag=f"qd{k}", name=f"qd{k}")
                           for k in range(20)]
                    for k in range(20):
                        for g in range(NG):
                            pt2 = psp.tile([128, GB * VO], BF, tag="ps", name="ps")
                            b0 = 64 * (g % 2)
                            nc.tensor.transpose(
                                pt2[:], nsl(Qn, g, GB * VO)
                                .rearrange("p c t -> p (c t)")[:, 128 * k:128 * (k + 1)],
                                ident[b0:b0 + GB * VO, b0:b0 + GB * VO])
                            nc.vector.tensor_copy(
                                QdC[k][:, g * GB * VO:(g + 1) * GB * VO], pt2[:])

                    sc_scale = float(1.0 / np.sqrt(np.float32(D)))
                    for g in range(NG):
                        qs = slice(g * GB * VO, (g + 1) * GB * VO)
                        ks = slice(g * GB * VK, (g + 1) * GB * VK)
                        qn_g = nsl(Qn, g, GB * VO).rearrange("p c t -> p (c t)")
                        on_g = nsl(On, g, GB * VO).rearrange("p c t -> p (c t)")
                        vf_g = Vf[:, g].rearrange("p c t -> p (c t)")
                        for h in range(NH):
                            pcs = _pieces(h * DS, (h + 1) * DS)
                            psc = psz.tile([GB * VO, GB * VK], F32, tag="psz", name="psz")
                            for pi, (ci, off, ln) in enumerate(pcs):
                                nc.tensor.matmul(psc[:], QdC[ci][off:off + ln, qs],
                                                 KfdC[ci][off:off + ln, ks],
                                                 start=(pi == 0),
                                                 stop=(pi == len(pcs) - 1))
                            E = work.tile([GB * VO, GB, VK], BF, tag="eat", name="eat")
                            nc.scalar.activation(E[:].rearrange("p a b -> p (a b)"),
                                                 psc[:], EXP, scale=sc_scale)
                            red = work.tile([GB * VO, GB], F32, tag="reda", name="reda")
                            nc.vector.tensor_reduce(red[:], E[:], mybir.AxisListType.X,
                                                    mybir.AluOpType.add)
                            rec = work.tile([GB * VO, GB], F32, tag="reca", name="reca")
                            nc.vector.reciprocal(rec[:], red[:])
                            AT = work.tile([GB * VO, GB * VK], BF, tag="at", name="at")
                            for j in range(GB):
                                nc.vector.tensor_scalar_mul(
                                    AT[:, j * VK:(j + 1) * VK], E[:, j], rec[:, j:j + 1])
                            nc.vector.tensor_mul(AT[:], AT[:], mask_att[:])
                            pat = psp.tile([GB * VK, GB * VO], BF, tag="ps", name="ps")
                            nc.tensor.transpose(pat[:], AT[:], ident[:GB * VO, :GB * VO])
                            ATT = work.tile([GB * VK, GB * VO], BF, tag="att", name="att")
                            nc.scalar.activation(ATT[:], pat[:], COPY)
                            pov = psz.tile([GB * VO, DS], F32, tag="psz", name="psz")
                            nc.tensor.matmul(pov[:], ATT[:],
                                             vf_g[:, h * DS:(h + 1) * DS],
                                             start=True, stop=True)
                            nc.vector.tensor_add(on_g[:, h * DS:(h + 1) * DS],
                                                 pov[:], qn_g[:, h * DS:(h + 1) * DS])

            # ---- fco on On ----
            with tc.tile_pool(name="fco", bufs=1) as fp:
                OnD = [fp.tile([128, BC * VO], BF, tag=f"ond{k}", name=f"ond{k}") for k in range(20)]
                for k in range(20):
                    for g in range(NG):
                        pt = psp.tile([128, GB * VO], BF, tag="ps", name="ps")
                        b0 = 64 * (g % 2)
                        for dt in range(2):
                            nc.tensor.transpose(pt[64 * dt:64 * dt + 64, :],
                                                nsl(On, g, GB * VO)[:, :, 2 * k + dt],
                                                ident[b0:b0 + GB * VO, b0:b0 + GB * VO])
                        nc.vector.tensor_copy(
                            OnD[k][:, g * GB * VO:(g + 1) * GB * VO], pt[:])

                def ores(g):
                    return nsl(On, g, GB * VO).rearrange("p c t -> p (c t)")

                Og = fp.tile([104, 4, C, T], BF, tag="Og", name="Og")

                def o_sink(g, n, yt):
                    ns = slice(n * 512, (n + 1) * 512)
                    nc.scalar.activation(
                        nsl(Og, g, GB * VO).rearrange("p c t -> p (c t)")[:, ns],
                        yt[:], RELU)

                run_gcn("o", OnD, VO, ores, o_sink)

                Fo = fp.tile([104, 4, C, T], BF, tag="Fo", name="Fo")
                for g in range(NG):
                    fo_g = nsl(Fo, g, GB * VO).rearrange("p c t -> p (c t)")
                    nc.vector.tensor_add(fo_g, ores(g),
                                         nsl(Og, g, GB * VO)
                                         .rearrange("p c t -> p (c t)"))
                    nc.sync.dma_start(out_d[g * GB * VO:(g + 1) * GB * VO, :], fo_g)
    nc.finalize()
    return nc


def _prep_consts(inp, sK, sQ):
    bf = ml_dtypes.bfloat16
    consts = {
        "sk": np.full((128, 1), sK, np.float32),
        "sq": np.full((128, 1), sQ, np.float32),
        "mask_att": np.kron(np.eye(GB, dtype=np.float32),
                            np.ones((VO, VK), np.float32)).astype(bf),
    }
    for gname, pref, Vn in (("k", "fck", VK), ("v", "fcv", VK), ("o", "fco", VO)):
        Wa = np.asarray(inp[pref + "_Wa"], np.float32) / (INTER * T)
        Wb = np.asarray(inp[pref + "_Wb"], np.float32)
        Wd = np.asarray(inp[pref + "_Wd"], np.float32)
        PA = np.asarray(inp[pref + "_PA"], np.float32)
        gam = np.asarray(inp[pref + "_gamma"], np.float32)
        bet = np.asarray(inp[pref + "_beta"], np.float32)
        gsc = gam / np.sqrt(np.float32(1.0 + 1e-5))
        Wdf = Wd * gsc[None, :, None]
        for i in range(3):
            wab = np.zeros((128, 64), np.float32)
            for dt in range(2):
                wab[dt * 64:dt * 64 + 64, dt * 16:dt * 16 + 16] = Wa[i].T
                wab[dt * 64:dt * 64 + 64, 32 + dt * 16:32 + dt * 16 + 16] = Wb[i].T
            consts[f"wab_{gname}{i}"] = wab.astype(bf)
        wd3 = np.zeros((128, 384), np.float32)
        for i in range(3):
            for dt in range(2):
                wd3[dt * 64:dt * 64 + 64,
                    i * 128 + dt * 64:i * 128 + dt * 64 + 64] = Wdf[i].T
        consts[f"wd_{gname}"] = wd3.astype(bf)
        mask = np.kron(np.eye(GB, dtype=np.float32), np.ones((Vn, Vn), np.float32))
        consts[f"mask_{gname}"] = mask.astype(bf)
        pam = np.stack([np.tile(PA[i].T, (GB, GB)) * mask for i in range(3)], axis=1)
        consts[f"pam_{gname}"] = pam.astype(bf)
        consts[f"beta_{gname}"] = np.repeat(bet, T).reshape(1, D).astype(bf)
    return consts


_cache = {}
_RUN = {}


def _make_runner(nc):
    # Cached-jit replication of bass_utils.run_bass_kernel_spmd's axon
    # path (bass2jax.run_bass_via_pjrt): identical NEFF + shard_map, but
    # the jit closure is built once so repeat calls skip re-trace and
    # BIR re-serialization (the 10k-instruction program costs ~1.5s/call
    # through the public wrapper on this 1-cpu host).
    import jax
    from jax.sharding import Mesh, PartitionSpec
    from concourse import bass2jax
    try:
        from jax.experimental.shard_map import shard_map
    except ImportError:
        from jax import shard_map
    bass2jax.install_neuronx_cc_hook()
    in_names, out_names, out_avals = [], [], []
    for alloc in nc.m.functions[0].allocations:
        if not isinstance(alloc, mybir.MemoryLocationSet):
            continue
        name = alloc.memorylocations[0].name
        if alloc.kind == "ExternalInput":
            in_names.append(name)
        elif alloc.kind == "ExternalOutput":
            out_names.append(name)
            out_avals.append(jax.core.ShapedArray(
                tuple(alloc.tensor_shape), mybir.dt.np(alloc.dtype)))
    pname = nc.partition_id_tensor.name if nc.partition_id_tensor else None
    if pname is not None:
        in_names.remove(pname)
    n_params = len(in_names)
    donate = tuple(range(n_params, n_params + len(out_names)))
    all_names = in_names + out_names + ([pname] if pname else [])

    def _body(*args):
        operands = list(args)
        if pname is not None:
            operands.append(bass2jax.partition_id_tensor())
        outs = bass2jax._bass_exec_p.bind(
            *operands, out_avals=tuple(out_avals),
            in_names=tuple(all_names), out_names=tuple(out_names),
            lowering_input_output_aliases=(), sim_require_finite=True,
            sim_require_nnan=True, nc=nc)
        return tuple(outs)

    mesh = Mesh(np.asarray(jax.devices()[:NCORES]), ("core",))
    ispec = (PartitionSpec("core"),) * (n_params + len(out_names))
    ospec = (PartitionSpec("core"),) * len(out_names)
    sharded = jax.jit(
        shard_map(_body, mesh=mesh, in_specs=ispec, out_specs=ospec,
                  check_rep=False),
        donate_argnums=donate, keep_unused=True)

    def run(by_name):
        zeros = [np.zeros((NCORES * a.shape[0], *a.shape[1:]), a.dtype)
                 for a in out_avals]
        outs = sharded(*[by_name[n] for n in in_names], *zeros)
        return {n: np.asarray(o) for n, o in zip(out_names, outs)}
    return run


def kernel(**inputs):
    Q = np.asarray(inputs["Q"], np.float32)
    K = np.asarray(inputs["K"], np.float32)
    B = Q.shape[0]
    sQ = float(np.abs(Q).max()) / 127.0
    sK = float(np.abs(K).max()) / 127.0
    Q8 = np.clip(np.rint(Q * (1.0 / sQ)), -127, 127).astype(np.int8)
    K8 = np.clip(np.rint(K * (1.0 / sK)), -127, 127).astype(np.int8)

    wkeys = sorted(k for k in inputs if k.startswith(("fck_", "fcv_", "fco_")))
    dig = hashlib.sha256()
    for k in wkeys:
        dig.update(np.ascontiguousarray(np.asarray(inputs[k])).tobytes())
    dig.update(np.float32(sQ).tobytes())
    dig.update(np.float32(sK).tobytes())
    key = dig.hexdigest()
    if key not in _cache:
        consts = _prep_consts(inputs, sK, sQ)
        _cache.clear()
        _RUN.clear()
        _cache[key] = _build(consts, NCORES)
    nc = _cache[key]
    if key not in _RUN:
        _RUN[key] = _make_runner(nc)

    out = _RUN[key]({"k8": K8.reshape(B * VK, D), "q8": Q8.reshape(B * VO, D)})
    return out["outp"].reshape(B, VO, D).astype(np.float32)


# revision 6
# speedup vs baseline: 2.5313x; 1.0319x over previous
"""nn_MAB kernel: 8-core data-parallel Bass/Tile implementation for TRN2.

The axon host<->device link (~40MB/s) dominates wall time, so inputs are
quantized to int8 on host (per-tensor scale) and the output returns as bf16.
Weights are host-folded (G-form not needed: a/b rank-16 transforms keep the
contraction at 640; 1/(inter*T) folded into Wa, gamma into Wd, biases are
structurally zero) and embedded in the NEFF as inline consts.

Device dataflow per core (32 batches, 8 groups of 4):
  k8 -> dequant bf16 -> PE-transpose to t-major d-chunks XD (p = dt*64+c).
  Per gcn, per subset: a/b transforms (full-width), M^T = b^T a per group
  (block-diag, off-diag garbage masked), free-dim softmax (no max-sub),
  +PA^T, PE-transpose -> S block-diag lhsT.  Then per group: U = Wd'
  transform, z = S^T U with beta seeded into PSUM via a K=1 ones matmul,
  y = relu(z + x) with x re-dequantized per group.
  Attention: c-major PE-transposes of Kf/Q, per-head scores (contraction
  pieces across 128-chunks), masked softmax, attn^T via PE, O = Q + attn@Vf.
"""
import hashlib
import numpy as np
import ml_dtypes

import concourse.bass as bass
from concourse import bacc, mybir, bass_utils, masks
from concourse.tile import TileContext

BF = mybir.dt.bfloat16
F32 = mybir.dt.float32
I8 = mybir.dt.int8
EXP = mybir.ActivationFunctionType.Exp
RELU = mybir.ActivationFunctionType.Relu
COPY = mybir.ActivationFunctionType.Copy

NCORES = 8
BC = 32
NG = 8
GB = 4
D, C, T, INTER = 2560, 64, 40, 16
VK, VO, NH, DS = 25, 10, 8, 320


def _pieces(lo, hi, step=128):
    out = []
    while lo < hi:
        ci, off = lo // step, lo % step
        ln = min(step - off, hi - lo)
        out.append((ci, off, ln))
        lo += ln
    return out


def _build(consts, num_devices):
    nc = bacc.Bacc("TRN2", target_bir_lowering=False, debug=False,
                   enable_asserts=False, num_devices=num_devices)
    k8_d = nc.dram_tensor("k8", (BC * VK, D), I8, kind="ExternalInput").ap()
    q8_d = nc.dram_tensor("q8", (BC * VO, D), I8, kind="ExternalInput").ap()
    out_d = nc.dram_tensor("outp", (BC * VO, D), BF, kind="ExternalOutput").ap()
    cap = {k: nc.inline_tensor(v, name="c_" + k).ap() for k, v in consts.items()}

    with TileContext(nc) as tc:
        with tc.tile_pool(name="persist", bufs=1) as persist, \
             tc.tile_pool(name="work", bufs=2) as work, \
             tc.tile_pool(name="ps", bufs=3, space="PSUM") as psp, \
             tc.tile_pool(name="psz", bufs=4, space="PSUM") as psz:

            ident = persist.tile([128, 128], BF, tag="ident", name="ident")
            masks.make_identity(nc, ident[:])

            def load_const(name, shape, dtype=BF):
                t = persist.tile(list(shape), dtype, tag="c_" + name)
                nc.sync.dma_start(t[:], cap[name])
                return t

            skc = load_const("sk", (128, 1), F32)
            sqc = load_const("sq", (128, 1), F32)
            ones_t = persist.tile([1, 128], BF, tag="ones", name="ones")
            nc.vector.memset(ones_t[:], 1.0)

            W = {}
            for g in ("k", "v", "o"):
                for i in range(3):
                    W[f"wab_{g}{i}"] = load_const(f"wab_{g}{i}", (128, 64))
                W[f"wd_{g}"] = load_const(f"wd_{g}", (128, 384))
                gv = VK if g != "o" else VO
                W[f"pam_{g}"] = load_const(f"pam_{g}", (GB * gv, 3, GB * gv))
                W[f"mask_{g}"] = load_const(f"mask_{g}", (GB * gv, GB * gv))
                W[f"beta_{g}"] = load_const(f"beta_{g}", (1, D))
            mask_att = load_const("mask_att", (GB * VO, GB * VK))

            # repacked 3-col layout for 40-row (VO) natural tensors
            def nsl(tile, g, R):
                return tile[64 * (g % 2):64 * (g % 2) + R, g // 2]

            On = persist.tile([104, 4, C, T], BF, tag="On", name="On")

            def dequant_k_group(pool, g, tag):
                st = work.tile([GB * VK, D], I8, tag="stg_k", name="stg_k")
                nc.sync.dma_start(st[:], k8_d[g * GB * VK:(g + 1) * GB * VK, :])
                xr = pool.tile([GB * VK, D], BF, tag=tag, name="kres")
                nc.vector.tensor_scalar_mul(xr[:], st[:], skc[:GB * VK])
                return xr

            def run_gcn(gname, XDl, Vn, res_ap, sink):
                """one unit_gcn.  XDl: 20 t-major chunks (128, BC*Vn).
                res_ap(g) -> (R, D) residual AP.  sink(g, n, yt) stores output."""
                R = GB * Vn
                W3 = BC * Vn
                with tc.tile_pool(name="gcn_" + gname, bufs=1) as gp, \
                     tc.tile_pool(name="gw_" + gname, bufs=1) as gw:
                    Sb = [[gp.tile([R, R], BF, tag=f"sb{i}_{g}", name=f"sb{i}_{g}") for g in range(NG)]
                          for i in range(3)]
                    nh2 = 2 if Vn == VK else 1
                    NW = W3 // nh2
                    for i in range(3):
                      for h2 in range(nh2):
                        Ad = [gw.tile([128, NW], BF, tag=f"ad{q}", name=f"ad{q}") for q in range(5)]
                        Bd = [gw.tile([128, NW], BF, tag=f"bd{q}", name=f"bd{q}") for q in range(5)]
                        wab = W[f"wab_{gname}{i}"]
                        for k in range(20):
                            pab = psp.tile([64, NW], F32, tag="ps", name="ps")
                            nc.tensor.matmul(pab[:], wab[:],
                                             XDl[k][:, h2 * NW:(h2 + 1) * NW],
                                             start=True, stop=True)
                            q, r0 = k // 4, 32 * (k % 4)
                            nc.vector.tensor_copy(Ad[q][r0:r0 + 32, :], pab[0:32, :])
                            nc.vector.tensor_copy(Bd[q][r0:r0 + 32, :], pab[32:64, :])
                        for g in range(h2 * NG // nh2, (h2 + 1) * NG // nh2):
                            gsl = slice(g * R - h2 * NW, (g + 1) * R - h2 * NW)
                            mt = psz.tile([R, R], F32, tag="psz", name="psz")
                            for q in range(5):
                                nc.tensor.matmul(mt[:], Bd[q][:, gsl], Ad[q][:, gsl],
                                                 start=(q == 0), stop=(q == 4))
                            E = work.tile([R, GB, Vn], BF, tag="et", name="et")
                            nc.scalar.activation(E[:].rearrange("p a b -> p (a b)"),
                                                 mt[:], EXP)
                            red = work.tile([R, GB], F32, tag="red", name="red")
                            nc.vector.tensor_reduce(red[:], E[:], mybir.AxisListType.X,
                                                    mybir.AluOpType.add)
                            rec = work.tile([R, GB], F32, tag="rec", name="rec")
                            nc.vector.reciprocal(rec[:], red[:])
                            ST = work.tile([R, R], BF, tag="st", name="st")
                            for j in range(GB):
                                nc.vector.tensor_scalar_mul(
                                    ST[:, j * Vn:(j + 1) * Vn], E[:, j], rec[:, j:j + 1])
                            nc.vector.tensor_mul(ST[:], ST[:], W[f"mask_{gname}"][:])
                            nc.vector.tensor_add(ST[:], ST[:], W[f"pam_{gname}"][:, i])
                            pst = psp.tile([R, R], BF, tag="ps", name="ps")
                            nc.tensor.transpose(pst[:], ST[:], ident[:R, :R])
                            nc.scalar.activation(Sb[i][g][:], pst[:], COPY)
                    for g in range(NG):
                        gs = slice(g * R, (g + 1) * R)
                        Ug = [gw.tile([R, C, T], BF, tag=f"ug{i}", name=f"ug{i}") for i in range(3)]
                        for k in range(20):
                            pu = psp.tile([R, 384], F32, tag="ps", name="ps")
                            nc.tensor.matmul(pu[:], XDl[k][:, gs], W[f"wd_{gname}"][:],
                                             start=True, stop=True)
                            for i in range(3):
                                dst = Ug[i][:, :, 2 * k:2 * k + 2].transpose([0, 2, 1])
                                src = pu[:, i * 128:i * 128 + 128] \
                                    .rearrange("p (a b) -> p a b", a=2)
                                nc.vector.tensor_copy(dst, src)
                        xres = res_ap(g)
                        for n in range(5):
                            ns = slice(n * 512, (n + 1) * 512)
                            pz = psz.tile([R, 512], F32, tag="psz", name="psz")
                            nc.tensor.matmul(pz[:], ones_t[:, :R],
                                             W[f"beta_{gname}"][:, ns],
                                             start=True, stop=False)
                            for i in range(3):
                                nc.tensor.matmul(
                                    pz[:], Sb[i][g][:],
                                    Ug[i][:].rearrange("p c t -> p (c t)")[:, ns],
                                    start=False, stop=(i == 2))
                            yt = work.tile([R, 512], BF, tag="yt", name="yt")
                            nc.vector.tensor_add(yt[:], pz[:], xres[:, ns])
                            sink(g, n, yt)

            with tc.tile_pool(name="kv", bufs=1) as kvp:
                Vf = kvp.tile([GB * VK, NG, C, T], BF, tag="Vf", name="Vf")
                KfdC = [kvp.tile([128, BC * VK], BF, tag=f"kfd{k}", name=f"kfd{k}")
                        for k in range(20)]

                with tc.tile_pool(name="xd", bufs=1) as xdp:
                    XD = [xdp.tile([128, BC * VK], BF, tag=f"xd{k}", name=f"xd{k}")
                          for k in range(20)]
                    for g in range(NG):
                        xr = dequant_k_group(work, g, "kres")
                        xv = xr[:].rearrange("p (c t) -> p c t", c=C)
                        for k in range(20):
                            pt = psp.tile([128, GB * VK], BF, tag="ps", name="ps")
                            for dt in range(2):
                                nc.tensor.transpose(pt[64 * dt:64 * dt + 64, :],
                                                    xv[:, :, 2 * k + dt],
                                                    ident[:GB * VK, :GB * VK])
                            nc.vector.tensor_copy(
                                XD[k][:, g * GB * VK:(g + 1) * GB * VK], pt[:])

                    def kres(g):
                        st = work.tile([GB * VK, D], I8, tag="stg_k", name="stg_k")
                        nc.sync.dma_start(st[:], k8_d[g * GB * VK:(g + 1) * GB * VK, :])
                        xr = work.tile([GB * VK, D], BF, tag="kres", name="kres")
                        nc.vector.tensor_scalar_mul(xr[:], st[:], skc[:GB * VK])
                        return xr[:]

                    def k_sink(g, n, yt):
                        gs = slice(g * GB * VK, (g + 1) * GB * VK)
                        tk = work.tile([GB * VK, 512], BF, tag="tk", name="tk")
                        nc.scalar.activation(tk[:], yt[:], RELU)
                        for j in range(4):
                            ptk = psp.tile([128, GB * VK], BF, tag="ps", name="ps")
                            nc.tensor.transpose(ptk[:], tk[:, 128 * j:128 * (j + 1)],
                                                ident[:GB * VK, :GB * VK])
                            nc.vector.tensor_copy(KfdC[4 * n + j][:, gs], ptk[:])

                    def v_sink(g, n, yt):
                        ns = slice(n * 512, (n + 1) * 512)
                        nc.scalar.activation(
                            Vf[:, g].rearrange("p c t -> p (c t)")[:, ns], yt[:], RELU)

                    run_gcn("k", XD, VK, kres, k_sink)
                    run_gcn("v", XD, VK, kres, v_sink)

                # ---- attention ----
                with tc.tile_pool(name="attn", bufs=1) as ap_:
                    Qn = ap_.tile([104, 4, C, T], BF, tag="Qn", name="Qn")
                    for g in range(NG):
                        stq = work.tile([GB * VO, D], I8, tag="stg_k", name="stg_q")
                        nc.sync.dma_start(stq[:], q8_d[g * GB * VO:(g + 1) * GB * VO, :])
                        nc.vector.tensor_scalar_mul(
                            nsl(Qn, g, GB * VO).rearrange("p c t -> p (c t)"),
                            stq[:], sqc[:GB * VO])
                    QdC = [ap_.tile([128, BC * VO], BF, tag=f"qd{k}", name=f"qd{k}")
                           for k in range(20)]
                    for k in range(20):
                        for g in range(NG):
                            pt2 = psp.tile([128, GB * VO], BF, tag="ps", name="ps")
                            b0 = 64 * (g % 2)
                            nc.tensor.transpose(
                                pt2[:], nsl(Qn, g, GB * VO)
                                .rearrange("p c t -> p (c t)")[:, 128 * k:128 * (k + 1)],
                                ident[b0:b0 + GB * VO, b0:b0 + GB * VO])
                            nc.vector.tensor_copy(
                                QdC[k][:, g * GB * VO:(g + 1) * GB * VO], pt2[:])

                    sc_scale = float(1.0 / np.sqrt(np.float32(D)))
                    for g in range(NG):
                        qs = slice(g * GB * VO, (g + 1) * GB * VO)
                        ks = slice(g * GB * VK, (g + 1) * GB * VK)
                        qn_g = nsl(Qn, g, GB * VO).rearrange("p c t -> p (c t)")
                        on_g = nsl(On, g, GB * VO).rearrange("p c t -> p (c t)")
                        vf_g = Vf[:, g].rearrange("p c t -> p (c t)")
                        for h in range(NH):
                            pcs = _pieces(h * DS, (h + 1) * DS)
                            psc = psz.tile([GB * VO, GB * VK], F32, tag="psz", name="psz")
                            for pi, (ci, off, ln) in enumerate(pcs):
                                nc.tensor.matmul(psc[:], QdC[ci][off:off + ln, qs],
                                                 KfdC[ci][off:off + ln, ks],
                                                 start=(pi == 0),
                                                 stop=(pi == len(pcs) - 1))
                            E = work.tile([GB * VO, GB, VK], BF, tag="eat", name="eat")
                            nc.scalar.activation(E[:].rearrange("p a b -> p (a b)"),
                                                 psc[:], EXP, scale=sc_scale)
                            red = work.tile([GB * VO, GB], F32, tag="reda", name="reda")
                            nc.vector.tensor_reduce(red[:], E[:], mybir.AxisListType.X,
                                                    mybir.AluOpType.add)
                            rec = work.tile([GB * VO, GB], F32, tag="reca", name="reca")
                            nc.vector.reciprocal(rec[:], red[:])
                            AT = work.tile([GB * VO, GB * VK], BF, tag="at", name="at")
                            for j in range(GB):
                                nc.vector.tensor_scalar_mul(
                                    AT[:, j * VK:(j + 1) * VK], E[:, j], rec[:, j:j + 1])
                            nc.vector.tensor_mul(AT[:], AT[:], mask_att[:])
                            pat = psp.tile([GB * VK, GB * VO], BF, tag="ps", name="ps")
                            nc.tensor.transpose(pat[:], AT[:], ident[:GB * VO, :GB * VO])
                            ATT = work.tile([GB * VK, GB * VO], BF, tag="att", name="att")
                            nc.scalar.activation(ATT[:], pat[:], COPY)
                            pov = psz.tile([GB * VO, DS], F32, tag="psz", name="psz")
                            nc.tensor.matmul(pov[:], ATT[:],
                                             vf_g[:, h * DS:(h + 1) * DS],
                                             start=True, stop=True)
                            nc.vector.tensor_add(on_g[:, h * DS:(h + 1) * DS],
                                                 pov[:], qn_g[:, h * DS:(h + 1) * DS])

            # ---- fco on On ----
            with tc.tile_pool(name="fco", bufs=1) as fp:
                OnD = [fp.tile([128, BC * VO], BF, tag=f"ond{k}", name=f"ond{k}") for k in range(20)]
                for k in range(20):
                    for g in range(NG):
                        pt = psp.tile([128, GB * VO], BF, tag="ps", name="ps")
                        b0 = 64 * (g % 2)
                        for dt in range(2):
                            nc.tensor.transpose(pt[64 * dt:64 * dt + 64, :],
                                                nsl(On, g, GB * VO)[:, :, 2 * k + dt],
                                                ident[b0:b0 + GB * VO, b0:b0 + GB * VO])
                        nc.vector.tensor_copy(
                            OnD[k][:, g * GB * VO:(g + 1) * GB * VO], pt[:])

                def ores(g):
                    return nsl(On, g, GB * VO).rearrange("p c t -> p (c t)")

                Og = fp.tile([104, 4, C, T], BF, tag="Og", name="Og")

                def o_sink(g, n, yt):
                    ns = slice(n * 512, (n + 1) * 512)
                    nc.scalar.activation(
                        nsl(Og, g, GB * VO).rearrange("p c t -> p (c t)")[:, ns],
                        yt[:], RELU)

                run_gcn("o", OnD, VO, ores, o_sink)

                Fo = fp.tile([104, 4, C, T], BF, tag="Fo", name="Fo")
                for g in range(NG):
                    fo_g = nsl(Fo, g, GB * VO).rearrange("p c t -> p (c t)")
                    nc.vector.tensor_add(fo_g, ores(g),
                                         nsl(Og, g, GB * VO)
                                         .rearrange("p c t -> p (c t)"))
                    nc.sync.dma_start(out_d[g * GB * VO:(g + 1) * GB * VO, :], fo_g)
    nc.finalize()
    return nc


def _prep_consts(inp, sK, sQ):
    bf = ml_dtypes.bfloat16
    consts = {
        "sk": np.full((128, 1), sK, np.float32),
        "sq": np.full((128, 1), sQ, np.float32),
        "mask_att": np.kron(np.eye(GB, dtype=np.float32),
                            np.ones((VO, VK), np.float32)).astype(bf),
    }
    for gname, pref, Vn in (("k", "fck", VK), ("v", "fcv", VK), ("o", "fco", VO)):
        Wa = np.asarray(inp[pref + "_Wa"], np.float32) / (INTER * T)
        Wb = np.asarray(inp[pref + "_Wb"], np.float32)
        Wd = np.asarray(inp[pref + "_Wd"], np.float32)
        PA = np.asarray(inp[pref + "_PA"], np.float32)
        gam = np.asarray(inp[pref + "_gamma"], np.float32)
        bet = np.asarray(inp[pref + "_beta"], np.float32)
        gsc = gam / np.sqrt(np.float32(1.0 + 1e-5))
        Wdf = Wd * gsc[None, :, None]
        for i in range(3):
            wab = np.zeros((128, 64), np.float32)
            for dt in range(2):
                wab[dt * 64:dt * 64 + 64, dt * 16:dt * 16 + 16] = Wa[i].T
                wab[dt * 64:dt * 64 + 64, 32 + dt * 16:32 + dt * 16 + 16] = Wb[i].T
            consts[f"wab_{gname}{i}"] = wab.astype(bf)
        wd3 = np.zeros((128, 384), np.float32)
        for i in range(3):
            for dt in range(2):
                wd3[dt * 64:dt * 64 + 64,
                    i * 128 + dt * 64:i * 128 + dt * 64 + 64] = Wdf[i].T
        consts[f"wd_{gname}"] = wd3.astype(bf)
        mask = np.kron(np.eye(GB, dtype=np.float32), np.ones((Vn, Vn), np.float32))
        consts[f"mask_{gname}"] = mask.astype(bf)
        pam = np.stack([np.tile(PA[i].T, (GB, GB)) * mask for i in range(3)], axis=1)
        consts[f"pam_{gname}"] = pam.astype(bf)
        consts[f"beta_{gname}"] = np.repeat(bet, T).reshape(1, D).astype(bf)
    return consts


_cache = {}
_RUN = {}


def _make_runner(nc):
    # Cached-jit replication of bass_utils.run_bass_kernel_spmd's axon
    # path (bass2jax.run_bass_via_pjrt): identical NEFF + shard_map, but
    # the jit closure is built once so repeat calls skip re-trace and
    # BIR re-serialization (the 10k-instruction program costs ~1.5s/call
    # through the public wrapper on this 1-cpu host).
    import jax
    from jax.sharding import Mesh, PartitionSpec
    from concourse import bass2jax
    try:
        from jax.experimental.shard_map import shard_map
    except ImportError:
        from jax import shard_map
    bass2jax.install_neuronx_cc_hook()
    in_names, out_names, out_avals = [], [], []
    for alloc in nc.m.functions[0].allocations:
        if not isinstance(alloc, mybir.MemoryLocationSet):
            continue
        name = alloc.memorylocations[0].name
        if alloc.kind == "ExternalInput":
            in_names.append(name)
        elif alloc.kind == "ExternalOutput":
            out_names.append(name)
            out_avals.append(jax.core.ShapedArray(
                tuple(alloc.tensor_shape), mybir.dt.np(alloc.dtype)))
    pname = nc.partition_id_tensor.name if nc.partition_id_tensor else None
    if pname is not None:
        in_names.remove(pname)
    n_params = len(in_names)
    donate = tuple(range(n_params, n_params + len(out_names)))
    all_names = in_names + out_names + ([pname] if pname else [])

    def _body(*args):
        operands = list(args)
        if pname is not None:
            operands.append(bass2jax.partition_id_tensor())
        outs = bass2jax._bass_exec_p.bind(
            *operands, out_avals=tuple(out_avals),
            in_names=tuple(all_names), out_names=tuple(out_names),
            lowering_input_output_aliases=(), sim_require_finite=True,
            sim_require_nnan=True, nc=nc)
        return tuple(outs)

    mesh = Mesh(np.asarray(jax.devices()[:NCORES]), ("core",))
    ispec = (PartitionSpec("core"),) * (n_params + len(out_names))
    ospec = (PartitionSpec("core"),) * len(out_names)
    sharded = jax.jit(
        shard_map(_body, mesh=mesh, in_specs=ispec, out_specs=ospec,
                  check_rep=False),
        donate_argnums=donate, keep_unused=True)

    def run(by_name):
        zeros = [np.zeros((NCORES * a.shape[0], *a.shape[1:]), a.dtype)
                 for a in out_avals]
        outs = sharded(*[by_name[n] for n in in_names], *zeros)
        return {n: np.asarray(o) for n, o in zip(out_names, outs)}
    return run


def kernel(**inputs):
    Q = np.asarray(inputs["Q"], np.float32)
    K = np.asarray(inputs["K"], np.float32)
    B = Q.shape[0]
    sQ = float(np.abs(Q).max()) / 127.0
    sK = float(np.abs(K).max()) / 127.0
    Q8 = np.clip(np.rint(Q * (1.0 / sQ)), -127, 127).astype(np.int8)
    K8 = np.clip(np.rint(K * (1.0 / sK)), -127, 127).astype(np.int8)

    wkeys = sorted(k for k in inputs if k.startswith(("fck_", "fcv_", "fco_")))
    dig = hashlib.sha256()
    for k in wkeys:
        dig.update(np.ascontiguousarray(np.asarray(inputs[k])).tobytes())
    dig.update(np.float32(sQ).tobytes())
    dig.update(np.float32(sK).tobytes())
    key = dig.hexdigest()
    if key not in _cache:
        consts = _prep_consts(inputs, sK, sQ)
        _cache.clear()
        _RUN.clear()
        _cache[key] = _build(consts, NCORES)
    nc = _cache[key]
    if key not in _RUN:
        _RUN[key] = _make_runner(nc)

    out = _RUN[key]({"k8": K8.reshape(B * VK, D), "q8": Q8.reshape(B * VO, D)})
    return out["outp"].reshape(B, VO, D).astype(np.float32)
